# revision 16
# baseline (speedup 1.0000x reference)
"""CrossEncoderReranker TRN2 Bass kernel (natural-layout, bf16).

reference computation:
    x = concat([mention_embs[mention_idx], candidate_embs], 1)   # [T, 2H]
    h = relu(x @ W1 + b1)                                        # [T, H]
    s = (h @ W2 + b2)[:, 0]                                      # [T]
    out = scatter(s -> [N, MAXK] at (mention_idx, col_idx)) + 0.5 * faiss
    out = concat([out, nota_col], 1)                             # [N, MAXK+1]

Device strategy (8-way data parallel over contiguous mention ranges):
  * Split x @ W1 = A[mention[t]] + cand[t] @ W1_bot.  The per-mention part
    A = memb @ W1_top + b1 is tiny (N rows); it is precomputed and expanded
    to per-pair rows host-side (A_pair[t] = A[mention[t]], bf16) and
    streamed in as a slab alongside the candidates.
  * PE computes ONLY the irreducible candidate matmul, in natural layout:
    psum[t, j] = sum_k candT[k, t] * W1_bot[k, j]  (t on partitions).
    Per 128-t block: 6 k-chunks x (512 + 256 j-cols) = 4608 streaming
    cycles; everything is bf16 (same PE rate as f32r, half the DMA).
  * DVE adds A_pair into psum's result (pre-relu), ACT applies relu, and
    one fused DVE tensor_tensor_reduce does s = sum_j h*W2[j] + b2 along
    the free axis — the W2 reduction costs zero PE cycles.
  * s lands [t-part, 1]; 32x32 DVE stream-transposes flip each scores
    chunk to flat-t order for the DRAM scratch, then the ragged->padded
    scatter runs as an indirect-DMA gather of overlapping 64-wide windows
    (row m starts at segment offset m), masked host-side 0/1, + 0.5*faiss.
"""

import sys

sys.path.insert(0, "/opt/trn_rl_repo")

from contextlib import ExitStack

import numpy as np

import concourse.bass as bass
import concourse.tile as tile
from concourse import mybir
from concourse.tile_rust import add_dep_helper

F32 = mybir.dt.float32
BF16 = mybir.dt.bfloat16
I32 = mybir.dt.int32
AF = mybir.ActivationFunctionType
ALU = mybir.AluOpType

N_CORES = 8
H = 768
P = 128
KC = H // P            # 6 k-chunks per 768
MAXK = 64
TT = 512               # candidate tile size (4 blocks of 128)
NB = TT // P           # t-blocks per tile
CHT = 8                # tiles per scores chunk
CH = CHT * TT          # scores per chunk (4096)


class SplitDrainTileContext(tile.TileContext):
    """The tail drain would carry one sync wait per logical proc; walrus caps
    sync waits per instruction. Absorb the global clock one proc at a time
    through SP NOPs (<=1 wait each), then emit the drain with a zero clock."""

    def _drain_and_barrier(self, tick_clock, wait_clock):
        from concourse.vector_clock import ScopedClock, VectorClock

        vals = list(tick_clock.global_clock)
        nprocs = len(vals)
        for q in range(nprocs):
            if not vals[q]:
                continue
            partial = [vals[p] if p == q else 0 for p in range(nprocs)]
            nop = self.nc.sync.nop()
            wait_clock.add_sem_waits(
                nop.ins, ScopedClock({None: VectorClock(partial)})
            )
        drain_inst = self.nc.sync.drain()
        wait_clock.add_sem_waits(
            drain_inst.ins, ScopedClock({None: VectorClock([0] * nprocs)})
        )
        self.nc.all_engine_barrier()
        popped = self.nc._tile_sem_poison_stack.pop()
        assert popped is self._sem_poison
        self.nc.clear_and_free_semaphores(list(self.sems.allocated().values()))
        self.nc.all_engine_barrier()


def split_waits(nc, cap=1):
    """This walrus build allows only ONE sync wait per instruction (two for
    some structs, but one is universally safe).  Move extra waits onto
    freshly inserted same-engine NOPs placed right before the instruction —
    the engine stalls at the NOP instead, semantics unchanged."""
    for fn in nc.m.functions:
        for bb in fn.blocks:
            new = []
            for inst in bb.instructions:
                si = inst.sync_info
                waits = list(si.on_wait) if si and si.on_wait else []
                if len(waits) > cap:
                    keep = waits[-cap:]
                    for k, wt in enumerate(waits[:-cap]):
                        nop = mybir.InstNoOp(
                            name=f"{inst.name}-wsp{k}",
                            engine=inst.engine,
                            ins=[], outs=[],
                            sync_info=mybir.SyncInfo(on_wait=[wt], on_update=[]),
                        )
                        nc.register_instruction(nop)
                        new.append(nop)
                    inst.sync_info = mybir.SyncInfo(
                        on_wait=keep, on_update=list(si.on_update or [])
                    )
                new.append(inst)
            bb.instructions = new


def build_program(T_pad, M_pad, gdep):
    """One SPMD Bass program shared by all cores.

    gdep[mc]: index of the scores-chunk DMA that must land before output
    chunk mc can be gathered (max over cores).
    """
    NT = -(-T_pad // TT)               # last tile may be partial
    MC = M_pad // P
    R = T_pad // P                     # 128-t score rows
    assert len(gdep) == MC

    nc = bass.Bass()

    candT = nc.dram_tensor("candT", [P, KC * T_pad], BF16, kind="ExternalInput")
    apair = nc.dram_tensor("apair", [T_pad, H], BF16, kind="ExternalInput")
    w1n = nc.dram_tensor("w1n", [P, KC * H], BF16, kind="ExternalInput")
    w2b = nc.dram_tensor("w2b", [P, H], BF16, kind="ExternalInput")
    b2b = nc.dram_tensor("b2b", [P, 1], F32, kind="ExternalInput")
    mask01 = nc.dram_tensor("mask01", [P, MC * MAXK], F32, kind="ExternalInput")
    fhalf = nc.dram_tensor("fhalf", [P, MC * MAXK], F32, kind="ExternalInput")
    offs = nc.dram_tensor("offs", [P, MC], I32, kind="ExternalInput")

    out = nc.dram_tensor("out", [M_pad, MAXK], F32, kind="ExternalOutput")
    # scores scratch, [R rows of 128] + 1 pad row for gather overread
    sc_dram = nc.dram_tensor("sc_scratch", [R + 1, P], F32, kind="Internal")

    with ExitStack() as ctx:
        tc = ctx.enter_context(SplitDrainTileContext(nc))
        cst = ctx.enter_context(tc.tile_pool(name="cst", bufs=1))
        candp = ctx.enter_context(tc.tile_pool(name="candp", bufs=3))
        app = ctx.enter_context(tc.tile_pool(name="app", bufs=3))
        qp = ctx.enter_context(tc.tile_pool(name="qp", bufs=3))
        gp = ctx.enter_context(tc.tile_pool(name="gp", bufs=2))
        scp = ctx.enter_context(tc.tile_pool(name="scp", bufs=2))
        sctp = ctx.enter_context(tc.tile_pool(name="sctp", bufs=2))
        hps = ctx.enter_context(tc.tile_pool(name="hps", bufs=4, space="PSUM"))

        # ---- constants ----
        w1_sb = cst.tile([P, KC * H], BF16)
        nc.sync.dma_start(w1_sb[:], w1n[:])
        w2_sb = cst.tile([P, H], BF16)
        nc.sync.dma_start(w2_sb[:], w2b[:])
        b2_sb = cst.tile([P, 1], F32)
        nc.sync.dma_start(b2_sb[:], b2b[:])
        mask_sb = cst.tile([P, MC * MAXK], F32)
        nc.sync.dma_start(mask_sb[:], mask01[:])
        fh_sb = cst.tile([P, MC * MAXK], F32)
        nc.sync.dma_start(fh_sb[:], fhalf[:])
        offs_sb = cst.tile([P, MC], I32)
        nc.sync.dma_start(offs_sb[:], offs[:])

        # zero the gather-overread pad row of the scores scratch
        z_t = cst.tile([1, P], F32)
        nc.vector.memset(z_t[:], 0.0)
        nc.sync.dma_start(sc_dram[R:R + 1, :], z_t[0:1, :])

        w1v = w1_sb[:].rearrange("p (k j) -> p k j", k=KC)

        # ---- output stage: ragged->padded gather + mask + faiss ----
        def emit_out_chunk(mc, sc_dma):
            g_t = gp.tile([P, MAXK], F32, tag="gath")
            gth = nc.gpsimd.indirect_dma_start(
                out=g_t[:], out_offset=None,
                in_=sc_dram[:].rearrange("r p -> (r p)")[:, None],
                in_offset=bass.IndirectOffsetOnAxis(ap=offs_sb[:, mc:mc + 1], axis=0),
            )
            add_dep_helper(gth.ins, sc_dma.ins, reason="gather needs scores")
            gm_t = gp.tile([P, MAXK], F32, tag="gm")
            nc.vector.tensor_tensor(
                gm_t[:], g_t[:], mask_sb[:, mc * MAXK:(mc + 1) * MAXK], ALU.mult
            )
            o_t = gp.tile([P, MAXK], F32, tag="osb")
            nc.vector.tensor_tensor(
                o_t[:], gm_t[:], fh_sb[:, mc * MAXK:(mc + 1) * MAXK], ALU.add
            )
            nc.sync.dma_start(out[mc * P:(mc + 1) * P, :], o_t[:])

        # ---- main loop over candidate tiles ----
        sch_t = None
        sc_dmas = []
        for i in range(NT):
            t0 = i * TT
            nb_i = min(NB, (T_pad - t0) // P)    # blocks in this tile
            tt_i = nb_i * P
            if i % CHT == 0:
                ci = i // CHT
                sch_t = scp.tile([P, CHT * NB], F32, tag="scchunk")
                if min(R - ci * CHT * NB, CHT * NB) < CHT * NB:
                    nc.vector.memset(sch_t[:], 0.0)
            cand_t = candp.tile([P, KC * TT], BF16, tag="cand")
            nc.sync.dma_start(
                cand_t[:].rearrange("p (k t) -> p k t", k=KC)[:, :, 0:tt_i],
                candT[:].rearrange("p (k t) -> p k t", k=KC)[:, :, t0:t0 + tt_i],
            )
            ap_t = app.tile([P, NB * H], BF16, tag="apair")
            nc.sync.dma_start(
                ap_t[:].rearrange("p (b j) -> p b j", b=NB)[:, 0:nb_i, :],
                apair[t0:t0 + tt_i, :].rearrange("(b p) j -> p b j", p=P),
            )
            cv = cand_t[:].rearrange("p (k t) -> p k t", k=KC)
            av = ap_t[:].rearrange("p (b j) -> p b j", b=NB)

            for b in range(nb_i):
                ps = hps.tile([P, 1024], F32, tag="hpsum")
                for (j0, jn) in ((0, 512), (512, 256)):
                    for kc in range(KC):
                        nc.tensor.matmul(
                            ps[:, j0:j0 + jn],
                            lhsT=cv[:, kc, b * P:(b + 1) * P],
                            rhs=w1v[:, kc, j0:j0 + jn],
                            start=(kc == 0), stop=(kc == KC - 1),
                        )
                # pre-relu mention part: q = psum + A_pair   (bf16 out)
                q_t = qp.tile([P, H], BF16, tag="q")
                nc.vector.tensor_tensor(
                    q_t[:], ps[:, 0:H], av[:, b, :], ALU.add
                )
                # fused relu + W2 stage: sch[:, c] = sum_j max(q,0)*w2
                c = (i % CHT) * NB + b
                nc.vector.scalar_tensor_tensor(
                    out=q_t[:],
                    in0=q_t[:], scalar=0.0, in1=w2_sb[:],
                    op0=ALU.max, op1=ALU.mult,
                    accum_out=sch_t[:, c:c + 1],
                )

            if i % CHT == CHT - 1 or i == NT - 1:
                ci = i // CHT
                nb_ch = (i % CHT) * NB + nb_i        # blocks in this chunk
                # + b2 on the whole chunk, then transpose
                nc.vector.tensor_scalar(
                    sch_t[:], sch_t[:], b2_sb[:, 0:1], None, ALU.add
                )
                # transpose [128, 32] -> [32, 128] via 32x32 stream transposes
                sct_t = sctp.tile([32, P], F32, tag="sct")
                for a in range(4):
                    nc.vector.transpose(
                        sct_t[0:32, a * 32:(a + 1) * 32],
                        sch_t[a * 32:(a + 1) * 32, 0:32],
                    )
                d = nc.sync.dma_start(
                    sc_dram[ci * 32:ci * 32 + nb_ch, :],
                    sct_t[0:nb_ch, :],
                )
                sc_dmas.append(d)
                for mc in range(MC):
                    if gdep[mc] == ci:
                        emit_out_chunk(mc, d)

    split_waits(nc)
    return nc


def prepare(inputs):
    """Shard + lay out the full inputs; returns (build params, in_maps, meta)."""
    import ml_dtypes

    bf16 = ml_dtypes.bfloat16

    mention_embs = np.asarray(inputs["mention_embs"], dtype=np.float32)
    candidate_embs = np.asarray(inputs["candidate_embs"], dtype=np.float32)
    W1 = np.asarray(inputs["W1"], dtype=np.float32)
    b1 = np.asarray(inputs["b1"], dtype=np.float32)
    W2 = np.asarray(inputs["W2"], dtype=np.float32)
    b2 = np.asarray(inputs["b2"], dtype=np.float32)
    faiss_prior = np.asarray(inputs["faiss_prior"], dtype=np.float32)
    mention_idx = np.asarray(inputs["mention_idx"], dtype=np.int64)
    col_idx = np.asarray(inputs["col_idx"], dtype=np.int64)

    N = mention_embs.shape[0]
    T = mention_idx.shape[0]
    assert np.all(np.diff(mention_idx) >= 0), "mention_idx must be sorted"
    lengths = np.bincount(mention_idx, minlength=N)
    offsets = np.concatenate([[0], np.cumsum(lengths)[:-1]])
    # col_idx must be arange within each contiguous segment
    assert np.array_equal(col_idx, np.arange(T) - np.repeat(offsets, lengths))

    # per-mention pre-relu part, exact in f32 host math
    A = mention_embs @ W1[:H] + b1                     # [N, H]

    # split mentions into 8 contiguous groups with ~equal candidate counts;
    # pick each boundary nearest the ideal cut to minimize max T_c
    cum = np.cumsum(lengths)
    bnd = [0]
    for c in range(1, N_CORES):
        tgt = c * T / N_CORES
        b = int(np.searchsorted(cum, tgt))
        if b + 1 < N and abs(cum[b] - tgt) > abs(cum[b - 1] - tgt):
            pass
        else:
            b = b + 1
        bnd.append(max(bnd[-1] + 1, min(b, N - (N_CORES - c))))
    bnd.append(N)

    T_cs = [int(cum[bnd[c + 1] - 1] - (cum[bnd[c] - 1] if bnd[c] else 0))
            for c in range(N_CORES)]
    M_cs = [bnd[c + 1] - bnd[c] for c in range(N_CORES)]
    T_pad = -(-max(T_cs) // P) * P                 # 128-granular
    M_pad = -(-max(M_cs) // P) * P
    MC = M_pad // P

    # gather dependency: which scores-chunk DMA must land before output
    # chunk mc can be gathered — max over cores
    NT = -(-T_pad // TT)
    n_chunks = (NT + CHT - 1) // CHT
    gdep = [0] * MC
    core_data = []
    for c in range(N_CORES):
        m0, m1 = bnd[c], bnd[c + 1]
        t0 = int(offsets[m0])
        offs_c = (offsets[m0:m1] - t0).astype(np.int64)
        core_data.append((m0, m1, t0, T_cs[c], M_cs[c], offs_c))
        for mc in range(MC):
            rows = offs_c[mc * P:(mc + 1) * P]
            if rows.size == 0:
                continue
            end = min(int(rows.max()) + MAXK, T_pad)
            k = min((end - 1) // CH, n_chunks - 1)
            gdep[mc] = max(gdep[mc], k)

    # shared (replicated) tensors
    w1n_l = np.ascontiguousarray(
        W1[H:].reshape(KC, P, H).transpose(1, 0, 2).reshape(P, KC * H)
    ).astype(bf16)
    w2b_l = np.tile(W2[:, 0][None, :], (P, 1)).astype(bf16)
    b2b_l = np.full((P, 1), b2[0], dtype=np.float32)

    in_maps = []
    for c in range(N_CORES):
        m0, m1, t0, T_c, M_c, offs_c = core_data[c]
        candT_l = np.zeros((P, KC * T_pad), dtype=bf16)
        cT = candidate_embs[t0:t0 + T_c].T.reshape(KC, P, T_c)
        for kc in range(KC):
            candT_l[:, kc * T_pad:kc * T_pad + T_c] = cT[kc]
        apair_l = np.zeros((T_pad, H), dtype=bf16)
        apair_l[:T_c] = A[mention_idx[t0:t0 + T_c]]
        lens_l = np.zeros(M_pad, dtype=np.int64)
        lens_l[:M_c] = lengths[m0:m1]
        mask_l = (np.arange(MAXK)[None, :] < lens_l[:, None]).astype(np.float32)
        fh_l = np.zeros((M_pad, MAXK), dtype=np.float32)
        fh_l[:M_c] = 0.5 * faiss_prior[m0:m1]
        offs_l = np.zeros(M_pad, dtype=np.int32)
        offs_l[:M_c] = offs_c.astype(np.int32)
        in_maps.append({
            "candT": candT_l,
            "apair": apair_l,
            "w1n": w1n_l, "w2b": w2b_l, "b2b": b2b_l,
            "mask01": np.ascontiguousarray(
                mask_l.reshape(MC, P, MAXK).transpose(1, 0, 2).reshape(P, MC * MAXK)),
            "fhalf": np.ascontiguousarray(
                fh_l.reshape(MC, P, MAXK).transpose(1, 0, 2).reshape(P, MC * MAXK)),
            "offs": np.ascontiguousarray(offs_l.reshape(MC, P).T),
        })
    return (T_pad, M_pad, gdep), in_maps, (bnd, N)


def assemble(results, meta, nota_bias):
    bnd, N = meta
    out = np.empty((N, MAXK + 1), dtype=np.float32)
    for c in range(N_CORES):
        m0, m1 = bnd[c], bnd[c + 1]
        out[m0:m1, :MAXK] = results[c]["out"][:m1 - m0]
    out[:, MAXK] = np.float32(nota_bias)
    return out


_CACHE = {}


def kernel(**inputs) -> np.ndarray:
    from concourse.bass_utils import run_bass_kernel_spmd

    (T_pad, M_pad, gdep), in_maps, meta = prepare(inputs)
    key = (T_pad, M_pad, tuple(gdep))
    if key not in _CACHE:
        _CACHE[key] = build_program(T_pad, M_pad, gdep)
    nc = _CACHE[key]
    res = run_bass_kernel_spmd(nc, in_maps, list(range(N_CORES)))
    return assemble(res.results, meta, np.asarray(inputs["nota_bias"]))
